# revision 23
# baseline (speedup 1.0000x reference)
"""Trainium2 Bass kernel for nn_ChannelWiseAttention (B=16, C=4096, d_model=384, H=6, Dh=64).

Strategy (data-parallel over B, 2 batches per core, 8 cores):
  device per core (8192 tokens):
    - qkv = x @ W.T + b   (fp32 TensorE matmuls, tokens on PSUM partitions)
    - cross-head scores S[n,h,g] = <q_nh, k_ng>  (GPSIMD multiply + DVE segment reduce)
    - ships v (biased) and S to DRAM
  host:
    - per-(n,h) argmax over g (k_heads=1 -> softmax over 1 element == select)
    - out[n, dh*6+h] = v[n, g*, dh]  (Dh-major layout, as in the reference)
    - channel importance + per-batch top-409 + gather
"""
import numpy as np

B, C, DM = 16, 4096, 384
H, DH = 6, 64
KCH = 409
NCORES = 8
TPC = B * C // NCORES          # tokens per core: 8192
NT = TPC // 128                # 64 tiles of 128 tokens

_CTX = {}


def _build():
    import concourse.tile as tile
    from concourse import bacc, mybir

    F32 = mybir.dt.float32
    F16 = mybir.dt.float16
    nc = bacc.Bacc("TRN2", target_bir_lowering=False, debug=False)
    XTH = nc.declare_dram_parameter("XTH", [384, TPC], F16, isOutput=False)
    XTL = nc.declare_dram_parameter("XTL", [384, TPC], F16, isOutput=False)
    WH3 = nc.declare_dram_parameter("WH3", [128, 3 * 1152], F16, isOutput=False)
    WL3 = nc.declare_dram_parameter("WL3", [128, 3 * 1152], F16, isOutput=False)
    BH = nc.declare_dram_parameter("BH", [1, 1152], F16, isOutput=False)
    BL = nc.declare_dram_parameter("BL", [1, 1152], F16, isOutput=False)
    V = nc.declare_dram_parameter("V", [TPC, 384], F32, isOutput=True)
    SC = nc.declare_dram_parameter("SC", [128, NT * 36], F32, isOutput=True)

    GSZ = 8                     # tiles per x-DMA group
    with tile.TileContext(nc) as tc:
        with (
            tc.tile_pool(name="const", bufs=1) as const,
            tc.tile_pool(name="xt", bufs=2) as xtp,
            tc.tile_pool(name="work", bufs=5) as work,
            tc.tile_pool(name="ps", bufs=2, space="PSUM") as ps,
            tc.tile_pool(name="psa", bufs=3, space="PSUM") as psa,
        ):
            wh3 = const.tile([128, 3 * 1152], F16)
            nc.sync.dma_start(out=wh3[:], in_=WH3[:])
            wl3 = const.tile([128, 3 * 1152], F16)
            nc.sync.dma_start(out=wl3[:], in_=WL3[:])
            bh = const.tile([1, 1152], F16)
            nc.sync.dma_start(out=bh[:], in_=BH[:])
            bl = const.tile([1, 1152], F16)
            nc.sync.dma_start(out=bl[:], in_=BL[:])
            ones = const.tile([1, 128], F16)
            nc.vector.memset(ones[:], 1.0)
            scores = const.tile([128, NT * 36], F32)

            for g0 in range(0, NT, GSZ):
                gw = GSZ * 128
                xth = xtp.tile([128, 3 * gw], F16)
                xtl = xtp.tile([128, 3 * gw], F16)
                for c in range(3):
                    nc.sync.dma_start(
                        out=xth[:, c * gw:(c + 1) * gw],
                        in_=XTH[c * 128:(c + 1) * 128, g0 * 128:g0 * 128 + gw],
                    )
                    nc.sync.dma_start(
                        out=xtl[:, c * gw:(c + 1) * gw],
                        in_=XTL[c * 128:(c + 1) * 128, g0 * 128:g0 * 128 + gw],
                    )
                for ti in range(GSZ):
                    t = g0 + ti
                    psq = psa.tile([128, 384], F32)
                    psk = ps.tile([128, 384], F32)
                    psv = ps.tile([128, 384], F32)
                    for blk, psb in enumerate((psq, psk, psv)):
                        lo, hi = blk * 384, (blk + 1) * 384
                        terms3 = blk == 2        # v needs full fp16x3 precision
                        for c in range(3):
                            xs = slice(c * gw + ti * 128, c * gw + (ti + 1) * 128)
                            ws = slice(c * 1152 + lo, c * 1152 + hi)
                            nc.tensor.matmul(
                                psb[:], xth[:, xs], wh3[:, ws],
                                start=(c == 0), stop=False,
                            )
                            if terms3:
                                nc.tensor.matmul(
                                    psb[:], xtl[:, xs], wh3[:, ws],
                                    start=False, stop=False,
                                )
                                nc.tensor.matmul(
                                    psb[:], xth[:, xs], wl3[:, ws],
                                    start=False, stop=False,
                                )
                        nc.tensor.matmul(
                            psb[:], ones[:], bh[:, lo:hi], start=False,
                            stop=not terms3,
                        )
                        if terms3:
                            nc.tensor.matmul(
                                psb[:], ones[:], bl[:, lo:hi], start=False, stop=True,
                            )
                    vout = work.tile([128, 384], F32)
                    nc.scalar.copy(vout[:], psv[:])
                    qkvg = work.tile([128, 640], F16)
                    nc.scalar.copy(qkvg[:, 0:384], psq[:])
                    nc.scalar.copy(qkvg[:, 384:640], psk[:, 0:256])
                    qkvb = work.tile([128, 512], F16)
                    nc.scalar.copy(qkvb[:, 0:384], psq[:])
                    nc.scalar.copy(qkvb[:, 384:512], psk[:, 256:384])
                    prodg = work.tile([128, 1536], F16)
                    q4 = qkvg[:, 0:384].unsqueeze(1).broadcast_to([128, 4, 384])
                    k4 = (qkvg[:, 384:640].rearrange("p (g d) -> p g d", g=4)
                          .unsqueeze(2).broadcast_to([128, 4, 6, 64]))
                    nc.gpsimd.tensor_mul(prodg[:], q4, k4)
                    prodd = work.tile([128, 768], F16)
                    q2 = qkvb[:, 0:384].unsqueeze(1).broadcast_to([128, 2, 384])
                    k2 = (qkvb[:, 384:512].rearrange("p (g d) -> p g d", g=2)
                          .unsqueeze(2).broadcast_to([128, 2, 6, 64]))
                    nc.vector.tensor_mul(prodd[:], q2, k2)
                    nc.vector.reduce_sum(
                        scores[:, t * 36:t * 36 + 24],
                        prodg[:].rearrange("p (gh d) -> p gh d", d=64),
                        axis=mybir.AxisListType.X,
                    )
                    nc.vector.reduce_sum(
                        scores[:, t * 36 + 24:t * 36 + 36],
                        prodd[:].rearrange("p (gh d) -> p gh d", d=64),
                        axis=mybir.AxisListType.X,
                    )
                    nc.sync.dma_start(
                        out=V[t * 128:(t + 1) * 128, :], in_=vout[:]
                    )
            nc.sync.dma_start(out=SC[:], in_=scores[:])
    nc.compile()
    return nc


def _get_nc():
    if "nc" not in _CTX:
        _CTX["nc"] = _build()
    return _CTX["nc"]


def _run_device(x, W, b):
    from concourse.bass_utils import run_bass_kernel_spmd

    nc = _get_nc()
    xt = np.ascontiguousarray(x.reshape(B * C, DM).T)            # [384, 65536]
    xth = xt.astype(np.float16)
    xtl = (xt - xth.astype(np.float32)).astype(np.float16)
    wt = np.ascontiguousarray(W.T)                               # [384, 1152]
    wth = wt.astype(np.float16)
    wtl = (wt - wth.astype(np.float32)).astype(np.float16)
    wh3 = np.concatenate([wth[c * 128:(c + 1) * 128] for c in range(3)], axis=1)
    wl3 = np.concatenate([wtl[c * 128:(c + 1) * 128] for c in range(3)], axis=1)
    bhv = b.astype(np.float16).reshape(1, 1152)
    blv = (b - bhv[0].astype(np.float32)).astype(np.float16).reshape(1, 1152)
    in_maps = [
        {
            "XTH": np.ascontiguousarray(xth[:, c * TPC:(c + 1) * TPC]),
            "XTL": np.ascontiguousarray(xtl[:, c * TPC:(c + 1) * TPC]),
            "WH3": wh3,
            "WL3": wl3,
            "BH": bhv,
            "BL": blv,
        }
        for c in range(NCORES)
    ]
    bkr = run_bass_kernel_spmd(
        nc, in_maps, list(range(NCORES)),
        trace=_CTX.get("trace", False), tmpdir=_CTX.get("tmpdir"),
    )
    _CTX["last_bkr"] = bkr
    results = bkr.results
    v = np.concatenate([r["V"] for r in results], axis=0)        # [65536, 384]
    sc = np.stack([r["SC"] for r in results], axis=0)            # [8, 128, NT*36]
    # SC layout per core: [p, t*36 + g*6 + h]; token n = t*128 + p
    sc = sc.reshape(NCORES, 128, NT, 6, 6).transpose(0, 2, 1, 3, 4)
    sc = sc.reshape(B * C, 6, 6)                                  # [n, g, h]
    return v, sc


EPS_SCORE = 3e-2    # head-argmax margins below this are re-derived in fp64
EPS_IMP = 1e-3      # importance-order margins below this are re-derived in fp64
TIE_WINDOW = 64     # sorted positions past KCH swept for near-boundary ties


def kernel(x, W, b):
    x = np.asarray(x, dtype=np.float32)
    W = np.asarray(W, dtype=np.float32)
    b = np.asarray(b, dtype=np.float32)
    v, sc = _run_device(x, W, b)

    N = B * C
    j = np.arange(384)
    hmap, dhmap = j % 6, j // 6

    gstar = np.argmax(sc, axis=1).astype(np.int64)               # [n, h] over g
    # out[n, dh*6 + h] = v[n, gstar[n,h]*64 + dh]   (Dh-major)
    cols = gstar[:, hmap] * 64 + dhmap[None, :]                  # [N, 384]
    out = np.take_along_axis(v, cols, axis=1)                    # [N, 384] fp32

    v64 = v.astype(np.float64)
    cols_sq = gstar * 64                                          # [N, H]
    normv = (v64.reshape(N, 6, 64) ** 2).sum(axis=2)              # [N, 6]
    imp2 = np.take_along_axis(normv, gstar, axis=1).sum(axis=1)   # [N]

    # --- tie hardening: decisions with tiny margins are re-derived in fp64 ---
    ssort = np.sort(sc, axis=1)                                   # over g
    score_gap = (ssort[:, -1, :] - ssort[:, -2, :]).min(axis=1)   # [N]
    tie = score_gap < EPS_SCORE

    imp2_b = imp2.reshape(B, C)
    order_full = np.argsort(-imp2_b, axis=1, kind="stable")
    win = KCH + TIE_WINDOW
    svals = np.take_along_axis(imp2_b, order_full[:, :win + 1], axis=1)
    close = (svals[:, :-1] - svals[:, 1:]) < EPS_IMP              # [B, win]
    for bi in range(B):
        ps = np.nonzero(close[bi])[0]
        chs = np.unique(np.concatenate([order_full[bi, ps], order_full[bi, ps + 1]]))
        tie[bi * C + chs] = True

    idx = np.nonzero(tie)[0]
    if idx.size:
        qkv64 = x.reshape(N, 384)[idx].astype(np.float64) @ W.astype(np.float64).T \
            + b.astype(np.float64)
        q64 = qkv64[:, 0:384].reshape(-1, 6, 64)
        k64 = qkv64[:, 384:768].reshape(-1, 6, 64)
        vv64 = qkv64[:, 768:1152]
        s64 = np.einsum("nhd,ngd->nhg", q64, k64)
        g64 = s64.argmax(axis=2)                                  # [n, h]
        c64 = g64[:, hmap] * 64 + dhmap[None, :]
        out[idx] = np.take_along_axis(vv64, c64, axis=1).astype(np.float32)
        nv64 = (vv64.reshape(-1, 6, 64) ** 2).sum(axis=2)
        imp2[idx] = np.take_along_axis(nv64, g64, axis=1).sum(axis=1)

    order = np.argsort(-imp2.reshape(B, C), axis=1, kind="stable")[:, :KCH]
    ch_idx = order.astype(np.int32)
    sparse_feat = np.take_along_axis(
        out.reshape(B, C, DM), order[:, :, None], axis=1
    ).astype(np.float32)
    return sparse_feat, ch_idx


# revision 24
# speedup vs baseline: 1.0246x; 1.0246x over previous
"""Trainium2 Bass kernel for nn_ChannelWiseAttention (B=16, C=4096, d_model=384, H=6, Dh=64).

Strategy (data-parallel over B, 2 batches per core, 8 cores):
  device per core (8192 tokens):
    - qkv = x @ W.T + b   (fp32 TensorE matmuls, tokens on PSUM partitions)
    - cross-head scores S[n,h,g] = <q_nh, k_ng>  (GPSIMD multiply + DVE segment reduce)
    - ships v (biased) and S to DRAM
  host:
    - per-(n,h) argmax over g (k_heads=1 -> softmax over 1 element == select)
    - out[n, dh*6+h] = v[n, g*, dh]  (Dh-major layout, as in the reference)
    - channel importance + per-batch top-409 + gather
"""
import numpy as np

B, C, DM = 16, 4096, 384
H, DH = 6, 64
KCH = 409
NCORES = 8
TPC = B * C // NCORES          # tokens per core: 8192
NT = TPC // 128                # 64 tiles of 128 tokens

_CTX = {}


def _build():
    import concourse.tile as tile
    from concourse import bacc, mybir

    F32 = mybir.dt.float32
    F16 = mybir.dt.float16
    nc = bacc.Bacc("TRN2", target_bir_lowering=False, debug=False)
    XTH = nc.declare_dram_parameter("XTH", [384, TPC], F16, isOutput=False)
    XTL = nc.declare_dram_parameter("XTL", [384, TPC], F16, isOutput=False)
    WH3 = nc.declare_dram_parameter("WH3", [128, 3 * 1152], F16, isOutput=False)
    WL3 = nc.declare_dram_parameter("WL3", [128, 3 * 1152], F16, isOutput=False)
    BH = nc.declare_dram_parameter("BH", [1, 1152], F16, isOutput=False)
    BL = nc.declare_dram_parameter("BL", [1, 1152], F16, isOutput=False)
    V = nc.declare_dram_parameter("V", [TPC, 384], F32, isOutput=True)
    SC = nc.declare_dram_parameter("SC", [128, NT * 36], F32, isOutput=True)

    GSZ = 8                     # tiles per x-DMA group
    with tile.TileContext(nc) as tc:
        with (
            tc.tile_pool(name="const", bufs=1) as const,
            tc.tile_pool(name="xt", bufs=2) as xtp,
            tc.tile_pool(name="work", bufs=4) as work,
            tc.tile_pool(name="ps", bufs=2, space="PSUM") as ps,
            tc.tile_pool(name="psa", bufs=3, space="PSUM") as psa,
        ):
            wh3 = const.tile([128, 3 * 1152], F16)
            nc.sync.dma_start(out=wh3[:], in_=WH3[:])
            wl3 = const.tile([128, 3 * 1152], F16)
            nc.sync.dma_start(out=wl3[:], in_=WL3[:])
            bh = const.tile([1, 1152], F16)
            nc.sync.dma_start(out=bh[:], in_=BH[:])
            bl = const.tile([1, 1152], F16)
            nc.sync.dma_start(out=bl[:], in_=BL[:])
            ones = const.tile([1, 128], F16)
            nc.vector.memset(ones[:], 1.0)
            scores = const.tile([128, NT * 36], F32)

            for g0 in range(0, NT, GSZ):
                gw = GSZ * 128
                xth = xtp.tile([128, 3 * gw], F16)
                xtl = xtp.tile([128, 3 * gw], F16)
                for c in range(3):
                    nc.sync.dma_start(
                        out=xth[:, c * gw:(c + 1) * gw],
                        in_=XTH[c * 128:(c + 1) * 128, g0 * 128:g0 * 128 + gw],
                    )
                    nc.sync.dma_start(
                        out=xtl[:, c * gw:(c + 1) * gw],
                        in_=XTL[c * 128:(c + 1) * 128, g0 * 128:g0 * 128 + gw],
                    )
                for ti in range(GSZ):
                    t = g0 + ti
                    psq = psa.tile([128, 384], F32)
                    psk = ps.tile([128, 384], F32)
                    psv = ps.tile([128, 384], F32)
                    for blk, psb in enumerate((psq, psk, psv)):
                        lo, hi = blk * 384, (blk + 1) * 384
                        terms3 = blk == 2        # v needs full fp16x3 precision
                        for c in range(3):
                            xs = slice(c * gw + ti * 128, c * gw + (ti + 1) * 128)
                            ws = slice(c * 1152 + lo, c * 1152 + hi)
                            nc.tensor.matmul(
                                psb[:], xth[:, xs], wh3[:, ws],
                                start=(c == 0), stop=False,
                            )
                            if terms3:
                                nc.tensor.matmul(
                                    psb[:], xtl[:, xs], wh3[:, ws],
                                    start=False, stop=False,
                                )
                                nc.tensor.matmul(
                                    psb[:], xth[:, xs], wl3[:, ws],
                                    start=False, stop=False,
                                )
                        nc.tensor.matmul(
                            psb[:], ones[:], bh[:, lo:hi], start=False,
                            stop=not terms3,
                        )
                        if terms3:
                            nc.tensor.matmul(
                                psb[:], ones[:], bl[:, lo:hi], start=False, stop=True,
                            )
                    vout = work.tile([128, 384], F32)
                    nc.scalar.copy(vout[:], psv[:])
                    qkvg = work.tile([128, 640], F16)
                    nc.scalar.copy(qkvg[:, 0:384], psq[:])
                    nc.scalar.copy(qkvg[:, 384:640], psk[:, 0:256])
                    qkvb = work.tile([128, 512], F16)
                    nc.scalar.copy(qkvb[:, 0:384], psq[:])
                    nc.scalar.copy(qkvb[:, 384:512], psk[:, 256:384])
                    prodg = work.tile([128, 1536], F16)
                    q4 = qkvg[:, 0:384].unsqueeze(1).broadcast_to([128, 4, 384])
                    k4 = (qkvg[:, 384:640].rearrange("p (g d) -> p g d", g=4)
                          .unsqueeze(2).broadcast_to([128, 4, 6, 64]))
                    nc.gpsimd.tensor_mul(prodg[:], q4, k4)
                    prodd = work.tile([128, 768], F16)
                    q2 = qkvb[:, 0:384].unsqueeze(1).broadcast_to([128, 2, 384])
                    k2 = (qkvb[:, 384:512].rearrange("p (g d) -> p g d", g=2)
                          .unsqueeze(2).broadcast_to([128, 2, 6, 64]))
                    nc.vector.tensor_mul(prodd[:], q2, k2)
                    nc.vector.reduce_sum(
                        scores[:, t * 36:t * 36 + 24],
                        prodg[:].rearrange("p (gh d) -> p gh d", d=64),
                        axis=mybir.AxisListType.X,
                    )
                    nc.vector.reduce_sum(
                        scores[:, t * 36 + 24:t * 36 + 36],
                        prodd[:].rearrange("p (gh d) -> p gh d", d=64),
                        axis=mybir.AxisListType.X,
                    )
                    nc.sync.dma_start(
                        out=V[t * 128:(t + 1) * 128, :], in_=vout[:]
                    )
            nc.sync.dma_start(out=SC[:], in_=scores[:])
    nc.compile()
    return nc


def _get_nc():
    if "nc" not in _CTX:
        _CTX["nc"] = _build()
    return _CTX["nc"]


def _run_device(x, W, b):
    from concourse.bass_utils import run_bass_kernel_spmd

    nc = _get_nc()
    xt = np.ascontiguousarray(x.reshape(B * C, DM).T)            # [384, 65536]
    xth = xt.astype(np.float16)
    xtl = (xt - xth.astype(np.float32)).astype(np.float16)
    wt = np.ascontiguousarray(W.T)                               # [384, 1152]
    wth = wt.astype(np.float16)
    wtl = (wt - wth.astype(np.float32)).astype(np.float16)
    wh3 = np.concatenate([wth[c * 128:(c + 1) * 128] for c in range(3)], axis=1)
    wl3 = np.concatenate([wtl[c * 128:(c + 1) * 128] for c in range(3)], axis=1)
    bhv = b.astype(np.float16).reshape(1, 1152)
    blv = (b - bhv[0].astype(np.float32)).astype(np.float16).reshape(1, 1152)
    in_maps = [
        {
            "XTH": np.ascontiguousarray(xth[:, c * TPC:(c + 1) * TPC]),
            "XTL": np.ascontiguousarray(xtl[:, c * TPC:(c + 1) * TPC]),
            "WH3": wh3,
            "WL3": wl3,
            "BH": bhv,
            "BL": blv,
        }
        for c in range(NCORES)
    ]
    bkr = run_bass_kernel_spmd(
        nc, in_maps, list(range(NCORES)),
        trace=_CTX.get("trace", False), tmpdir=_CTX.get("tmpdir"),
    )
    _CTX["last_bkr"] = bkr
    results = bkr.results
    v = np.concatenate([r["V"] for r in results], axis=0)        # [65536, 384]
    sc = np.stack([r["SC"] for r in results], axis=0)            # [8, 128, NT*36]
    # SC layout per core: [p, t*36 + g*6 + h]; token n = t*128 + p
    sc = sc.reshape(NCORES, 128, NT, 6, 6).transpose(0, 2, 1, 3, 4)
    sc = sc.reshape(B * C, 6, 6)                                  # [n, g, h]
    return v, sc


EPS_SCORE = 3e-2    # head-argmax margins below this are re-derived in fp64
EPS_IMP = 1e-3      # importance-order margins below this are re-derived in fp64
TIE_WINDOW = 64     # sorted positions past KCH swept for near-boundary ties


def kernel(x, W, b):
    x = np.asarray(x, dtype=np.float32)
    W = np.asarray(W, dtype=np.float32)
    b = np.asarray(b, dtype=np.float32)
    v, sc = _run_device(x, W, b)

    N = B * C
    j = np.arange(384)
    hmap, dhmap = j % 6, j // 6

    gstar = np.argmax(sc, axis=1).astype(np.int64)               # [n, h] over g
    # out[n, dh*6 + h] = v[n, gstar[n,h]*64 + dh]   (Dh-major)
    cols = gstar[:, hmap] * 64 + dhmap[None, :]                  # [N, 384]
    out = np.take_along_axis(v, cols, axis=1)                    # [N, 384] fp32

    v64 = v.astype(np.float64)
    cols_sq = gstar * 64                                          # [N, H]
    normv = (v64.reshape(N, 6, 64) ** 2).sum(axis=2)              # [N, 6]
    imp2 = np.take_along_axis(normv, gstar, axis=1).sum(axis=1)   # [N]

    # --- tie hardening: decisions with tiny margins are re-derived in fp64 ---
    ssort = np.sort(sc, axis=1)                                   # over g
    score_gap = (ssort[:, -1, :] - ssort[:, -2, :]).min(axis=1)   # [N]
    tie = score_gap < EPS_SCORE

    imp2_b = imp2.reshape(B, C)
    order_full = np.argsort(-imp2_b, axis=1, kind="stable")
    win = KCH + TIE_WINDOW
    svals = np.take_along_axis(imp2_b, order_full[:, :win + 1], axis=1)
    close = (svals[:, :-1] - svals[:, 1:]) < EPS_IMP              # [B, win]
    for bi in range(B):
        ps = np.nonzero(close[bi])[0]
        chs = np.unique(np.concatenate([order_full[bi, ps], order_full[bi, ps + 1]]))
        tie[bi * C + chs] = True

    idx = np.nonzero(tie)[0]
    if idx.size:
        qkv64 = x.reshape(N, 384)[idx].astype(np.float64) @ W.astype(np.float64).T \
            + b.astype(np.float64)
        q64 = qkv64[:, 0:384].reshape(-1, 6, 64)
        k64 = qkv64[:, 384:768].reshape(-1, 6, 64)
        vv64 = qkv64[:, 768:1152]
        s64 = np.einsum("nhd,ngd->nhg", q64, k64)
        g64 = s64.argmax(axis=2)                                  # [n, h]
        c64 = g64[:, hmap] * 64 + dhmap[None, :]
        out[idx] = np.take_along_axis(vv64, c64, axis=1).astype(np.float32)
        nv64 = (vv64.reshape(-1, 6, 64) ** 2).sum(axis=2)
        imp2[idx] = np.take_along_axis(nv64, g64, axis=1).sum(axis=1)

    order = np.argsort(-imp2.reshape(B, C), axis=1, kind="stable")[:, :KCH]
    ch_idx = order.astype(np.int32)
    sparse_feat = np.take_along_axis(
        out.reshape(B, C, DM), order[:, :, None], axis=1
    ).astype(np.float32)
    return sparse_feat, ch_idx
